# revision 6
# baseline (speedup 1.0000x reference)
"""Trainium2 Bass kernel for nn_ExampleModel_1116691497724 (moe_routing).

Math: the reference returns log_softmax_T( sum_D(moe_out) ), and sum_D
collapses the expert FFN to a dot product:
    sum_d (h @ W2[e] + b2[e]) = h . w2sum[e] + sum(b2[e]),  w2sum[e] = W2[e] @ 1
    (x @ W1[e] + b1[e]) . w2sum[e] = x . v[e] + c[e]
with v[e] = W1[e] @ w2sum[e]  (a [D] vector) and scalar
c[e] = b1[e].w2sum[e] + sum(b2[e]).  Then per token:
    s_e = x . v[e] + c[e],  logits = x @ Wg
    moe_sum = max(softmax(logits)) * s_argmax(logits)
    out = log_softmax over tokens (per batch row) of moe_sum.

Distribution over 8 cores:
  phase A (expert-parallel over H): core c reduces W2[:, 128c:128c+128, :] and
    computes the partial v from the matching W1 columns; one 16KB AllReduce
    sums partials so every core holds full (v, c).
  phase B (token-parallel): core c owns batch row c%4 (512 tokens): logits/s
    via PE (stationary [Wg|v] block weights, tokens streaming), gate+select
    via DVE/ACT after a PE transpose, local row log_softmax.  Host takes rows
    from cores 0..3.

Matmul orientation notes: lhsT (stationary) operands are kept tiny (M<=4
columns) so LDWEIGHTS is negligible; fp32 streams at 4 cycles/row.  The
d axis is decomposed as d = p*16 + n (partition-major) so that v arriving
from the AllReduce as a flat [2048] row loads into [128,16] tiles with
contiguous 64B per-partition runs.
"""

import sys

import numpy as np

for _p in ("/opt/trn_rl_repo",):
    if _p not in sys.path:
        sys.path.append(_p)

import concourse.bass as bass  # noqa: E402
import concourse.mybir as mybir  # noqa: E402
import concourse.tile as tile  # noqa: E402
from concourse import bacc, bass_utils  # noqa: E402
from concourse.masks import make_identity  # noqa: E402

# Problem shape (hardcoded per spec).
B, T, D, H, E = 4, 512, 2048, 1024, 2
P = 128
NCORES = 8
TB = T  # tokens per core = one batch row
NB = D // P  # 16 d-blocks
HC = H // NCORES  # 128 h-chunk per expert per core
NG = TB // P  # 4 token groups per core
DC = D // NCORES  # 256 b2 columns per core
VK = 4  # v computed in VK chunks of D/VK columns
F32 = mybir.dt.float32
AX = mybir.AxisListType
AF = mybir.ActivationFunctionType
ALU = mybir.AluOpType

PAY = 2 * D + 8  # AllReduce payload: v0 | v1 | c0 c1 | pad


def emit_kernel(nc, tc, io):
    """Emit the per-core program. io: dict of dram APs."""
    xt, w1t, w2r, wgt, b1c, b2c, out = (
        io["xt"], io["w1t"], io["w2r"], io["wgt"], io["b1c"], io["b2c"], io["out"],
    )
    with (
        tc.tile_pool(name="main", bufs=1) as pool,
        tc.tile_pool(name="psum", bufs=1, space="PSUM") as psum,
        tc.tile_pool(name="dram", bufs=1, space="DRAM") as dram,
    ):
        # ---- loads: weight chunks first (phase A critical path), x last ----
        w2_sb = pool.tile([P, E, D], F32)
        for e in range(E):
            nc.sync.dma_start(w2_sb[:, e, :], w2r[e])
        w1_sb = pool.tile([P, E, D], F32)
        DK = D // VK
        for e in range(E):
            for k in range(VK):
                nc.sync.dma_start(
                    w1_sb[:, e, k * DK : (k + 1) * DK], w1t[e, :, k * DK : (k + 1) * DK]
                )
        b1_sb = pool.tile([P, E], F32)
        nc.sync.dma_start(b1_sb[:], b1c)
        b2_sb = pool.tile([1, E * DC], F32)
        nc.sync.dma_start(b2_sb[:], b2c)
        wgt_sb = pool.tile([P, NB * E], F32)
        nc.sync.dma_start(wgt_sb[:], wgt)
        # phase B data: d-interleaved (d = p*16 + n), needed only after the AR
        x_sb = pool.tile([P, NB, TB], F32)
        nc.sync.dma_start(x_sb[:], xt.rearrange("(p n) t -> p n t", p=P))

        # ---- phase A: w2sum, then v chunks with w2sum as stationary weights ----
        w2s = pool.tile([P, E], F32)
        for e in range(E):
            nc.vector.reduce_sum(w2s[:, e : e + 1], w2_sb[:, e, :], axis=AX.X)
        b2s = pool.tile([1, E], F32)
        for e in range(E):
            nc.vector.reduce_sum(
                b2s[0:1, e : e + 1], b2_sb[0:1, e * DC : (e + 1) * DC], axis=AX.X
            )

        pay = pool.tile([1, PAY], F32)
        nc.gpsimd.memset(pay[0:1, 2 * D + 2 : PAY], 0.0)
        b1dot = psum.tile([1, E], F32)
        for e in range(E):
            for k in range(VK):
                vch = psum.tile([1, DK], F32, name="vch", tag="vch", bufs=2)
                nc.tensor.matmul(
                    vch[:],
                    w2s[:, e : e + 1],
                    w1_sb[:, e, k * DK : (k + 1) * DK],
                    start=True,
                    stop=True,
                )
                dst = pay[0:1, e * D + k * DK : e * D + (k + 1) * DK]
                if k % 2 == 0:
                    nc.vector.tensor_copy(dst, vch[:])
                else:
                    nc.scalar.copy(dst, vch[:])
            nc.tensor.matmul(
                b1dot[0:1, e : e + 1],
                w2s[:, e : e + 1],
                b1_sb[:, e : e + 1],
                start=True,
                stop=True,
            )
            nc.vector.tensor_add(
                pay[0:1, 2 * D + e : 2 * D + e + 1],
                b1dot[0:1, e : e + 1],
                b2s[0:1, e : e + 1],
            )

        arin = dram.tile([1, PAY], F32)
        arout = dram.tile([1, PAY], F32, addr_space="Shared")
        nc.sync.dma_start(arin[:], pay[:])
        nc.gpsimd.collective_compute(
            "AllReduce",
            ALU.add,
            replica_groups=[list(range(NCORES))],
            ins=[arin[:].opt()],
            outs=[arout[:].opt()],
        )

        # ---- phase B: [l0 l1 s0 s1] = m4-block.T @ x-block, accumulated ----
        vsb = pool.tile([P, E, NB], F32)
        nc.sync.dma_start(vsb[:], arout[0:1, 0 : 2 * D].rearrange("x (e p n) -> p (x e) n", p=P, e=E))
        crow = pool.tile([1, 4], F32)
        nc.gpsimd.memset(crow[:], 0.0)
        nc.sync.dma_start(crow[0:1, 2:4], arout[0:1, 2 * D : 2 * D + 2])
        m4 = pool.tile([P, NB, 4], F32)
        nc.vector.tensor_copy(m4[:, :, 0:2], wgt_sb.rearrange("p (n e) -> p n e", e=E))
        for e in range(E):
            nc.vector.tensor_copy(m4[:, :, 2 + e : 3 + e], vsb[:, e, :, None])
        ones_row = pool.tile([1, TB], F32)
        nc.gpsimd.memset(ones_row[:], 1.0)
        ident = pool.tile([4, 4], F32)
        make_identity(nc, ident[:])

        ps4 = psum.tile([4, TB], F32)
        for n in range(NB):
            nc.tensor.matmul(
                ps4[:], m4[:, n, :], x_sb[:, n, :], start=(n == 0), stop=False
            )
        nc.tensor.matmul(ps4[:], crow[0:1, :], ones_row[0:1, :], start=False, stop=True)
        sb4 = pool.tile([4, TB], F32)
        nc.vector.tensor_copy(sb4[:], ps4[:])

        moe_sb = pool.tile([P, NG], F32)
        for g in range(NG):
            tp = psum.tile([P, 4], F32, name=f"tp_{g}", tag="tp", bufs=2)
            nc.tensor.transpose(tp[:], sb4[0:4, g * P : (g + 1) * P], ident[:])
            # gate/select per token (rows): cols l0 l1 s0 s1
            t4 = pool.tile([P, 4], F32, name=f"t4_{g}")
            nc.vector.tensor_copy(t4[:], tp[:])
            negm = pool.tile([P, 1], F32, name=f"negm_{g}")
            nc.vector.reduce_max(negm[:], t4[:, 0:2], axis=AX.X, negate=True)
            z = pool.tile([P, E], F32, name=f"z_{g}")
            den = pool.tile([P, 1], F32, name=f"den_{g}")
            nc.scalar.activation(z[:], t4[:, 0:2], AF.Exp, bias=negm[:], accum_out=den[:])
            rec = pool.tile([P, 1], F32, name=f"rec_{g}")
            nc.vector.reciprocal(rec[:], den[:])
            zmax = pool.tile([P, 1], F32, name=f"zmax_{g}")
            nc.vector.reduce_max(zmax[:], z[:], axis=AX.X)
            gate = pool.tile([P, 1], F32, name=f"gate_{g}")
            nc.vector.tensor_mul(gate[:], zmax[:], rec[:])
            mask = pool.tile([P, 1], F32, name=f"mask_{g}")
            nc.vector.tensor_tensor(mask[:], t4[:, 0:1], t4[:, 1:2], op=ALU.is_ge)
            sdiff = pool.tile([P, 1], F32, name=f"sdiff_{g}")
            nc.vector.tensor_sub(sdiff[:], t4[:, 2:3], t4[:, 3:4])
            ssel = pool.tile([P, 1], F32, name=f"ssel_{g}")
            nc.vector.tensor_mul(ssel[:], mask[:], sdiff[:])
            nc.vector.tensor_add(ssel[:], ssel[:], t4[:, 3:4])
            nc.vector.tensor_mul(moe_sb[:, g : g + 1], gate[:], ssel[:])

        # ---- row log_softmax over all 512 tokens ----
        row = pool.tile([1, TB], F32)
        for g in range(NG):
            nc.sync.dma_start(row[0:1, g * P : (g + 1) * P], moe_sb[:, g : g + 1])
        negm2 = pool.tile([1, 1], F32)
        nc.vector.reduce_max(negm2[:], row[0:1, :], axis=AX.X, negate=True)
        expv = pool.tile([1, TB], F32)
        ssum = pool.tile([1, 1], F32)
        nc.scalar.activation(expv[:], row[0:1, :], AF.Exp, bias=negm2[:], accum_out=ssum[:])
        logs = pool.tile([1, 1], F32)
        nc.scalar.activation(logs[:], ssum[:], AF.Ln)
        shift = pool.tile([1, 1], F32)
        nc.vector.tensor_sub(shift[:], negm2[:], logs[:])
        res = pool.tile([1, TB], F32)
        nc.vector.tensor_scalar_add(res[:], row[0:1, :], shift[:])
        nc.sync.dma_start(out[:], res[:])


_CACHED = None


def build_program():
    global _CACHED
    if _CACHED is not None:
        return _CACHED
    nc = bacc.Bacc(
        "TRN2",
        target_bir_lowering=False,
        debug=False,
        enable_asserts=False,
        num_devices=NCORES,
    )
    io = {
        "xt": nc.dram_tensor("xt", [D, TB], F32, kind="ExternalInput").ap(),
        "w1t": nc.dram_tensor("w1t", [E, HC, D], F32, kind="ExternalInput").ap(),
        "w2r": nc.dram_tensor("w2r", [E, HC, D], F32, kind="ExternalInput").ap(),
        "wgt": nc.dram_tensor("wgt", [P, NB * E], F32, kind="ExternalInput").ap(),
        "b1c": nc.dram_tensor("b1c", [HC, E], F32, kind="ExternalInput").ap(),
        "b2c": nc.dram_tensor("b2c", [1, E * DC], F32, kind="ExternalInput").ap(),
        "out": nc.dram_tensor("out", [1, TB], F32, kind="ExternalOutput").ap(),
    }
    with tile.TileContext(nc) as tc:
        emit_kernel(nc, tc, io)
    nc.compile()
    _CACHED = nc
    return nc


def shard_inputs(x, Wg, W1, b1, W2, b2):
    """Host-side sharding: per-core input maps (layout prep only, no math)."""
    x = np.asarray(x, np.float32).reshape(B * T, D)
    Wg = np.asarray(Wg, np.float32)
    W1 = np.asarray(W1, np.float32)
    b1 = np.asarray(b1, np.float32)
    W2 = np.asarray(W2, np.float32)
    b2 = np.asarray(b2, np.float32)
    # wgt[p, n*2+e] = Wg[p*16+n, e]  (d = p*16 + n decomposition)
    wgt = np.ascontiguousarray(Wg.reshape(P, NB * E))
    in_maps = []
    for c in range(NCORES):
        row = c % B
        hs, he = c * HC, (c + 1) * HC
        in_maps.append(
            {
                "xt": np.ascontiguousarray(x[row * TB : (row + 1) * TB, :].T),
                "w1t": np.ascontiguousarray(W1[:, :, hs:he].transpose(0, 2, 1)),
                "w2r": np.ascontiguousarray(W2[:, hs:he, :]),
                "wgt": wgt,
                "b1c": np.ascontiguousarray(b1[:, hs:he].T),
                "b2c": np.ascontiguousarray(
                    b2[:, c * DC : (c + 1) * DC].reshape(1, E * DC)
                ),
            }
        )
    return in_maps


def run(in_maps, **kwargs):
    nc = build_program()
    return bass_utils.run_bass_kernel_spmd(
        nc, in_maps, core_ids=list(range(NCORES)), **kwargs
    )


def kernel(x, Wg, W1, b1, W2, b2):
    res = run(shard_inputs(x, Wg, W1, b1, W2, b2))
    return np.concatenate([res.results[b]["out"] for b in range(B)], axis=0)


# revision 12
# speedup vs baseline: 1.0042x; 1.0042x over previous
"""Trainium2 Bass kernel for nn_ExampleModel_1116691497724 (moe_routing).

Math: the reference returns log_softmax_T( sum_D(moe_out) ), and sum_D
collapses the expert FFN to a dot product:
    sum_d (h @ W2[e] + b2[e]) = h . w2sum[e] + sum(b2[e]),  w2sum[e] = W2[e] @ 1
    (x @ W1[e] + b1[e]) . w2sum[e] = x . v[e] + c[e]
with v[e] = W1[e] @ w2sum[e]  (a [D] vector) and scalar
c[e] = b1[e].w2sum[e] + sum(b2[e]).  Then per token:
    s_e = x . v[e] + c[e],  logits = x @ Wg
    moe_sum = max(softmax(logits)) * s_argmax(logits)
    out = log_softmax over tokens (per batch row) of moe_sum.

Distribution over 8 cores:
  phase A (expert-parallel over H): core c reduces W2[:, 128c:128c+128, :] and
    computes the partial v from the matching W1 columns; one 16KB collective
    combines partials so every core holds full (v, c).
  phase B (token-parallel): core c owns batch row c%4 (512 tokens).  The
    logits stream (fp32, exact — argmax ties must match the reference) runs in
    the collective's shadow; the s stream (f32r — no selection depends on it)
    runs after.  Gate/select per token after a PE transpose; local row
    log_softmax.  Host takes rows from cores 0..3.

Scheduling notes: stationary matmul operands are kept tiny (M<=2 columns) so
LDWEIGHTS is negligible; fp32 streams at 4 cycles/row, f32r at 1.  The x load
is FIFO-ordered behind the AllReduce payload store so the weight path (which
gates the collective) gets the DMA bandwidth first.
"""

import sys

import numpy as np

for _p in ("/opt/trn_rl_repo",):
    if _p not in sys.path:
        sys.path.append(_p)

import concourse.bass as bass  # noqa: E402
import concourse.mybir as mybir  # noqa: E402
import concourse.tile as tile  # noqa: E402
from concourse import bacc, bass_utils  # noqa: E402
from concourse.masks import make_identity  # noqa: E402

# Problem shape (hardcoded per spec).
B, T, D, H, E = 4, 512, 2048, 1024, 2
P = 128
NCORES = 8
TB = T  # tokens per core = one batch row
NB = D // P  # 16 d-blocks
HC = H // NCORES  # 128 h-chunk per expert per core
NG = TB // P  # 4 token groups per core
DC = D // NCORES  # 256 b2 columns per core
VK = 4  # v computed in VK chunks of D/VK columns
F32 = mybir.dt.float32
F32R = mybir.dt.float32r
AX = mybir.AxisListType
AF = mybir.ActivationFunctionType
ALU = mybir.AluOpType

PAY = 2 * D + 8  # collective payload: v0 | v1 | c0 c1 | pad

COLLECTIVE = "AG"  # "AG": AllGather + local sum; "AR": AllReduce
F32R_S = True  # s stream in f32r (inputs rounded via cast copies in the AR shadow)


def emit_kernel(nc, tc, io):
    """Emit the per-core program. io: dict of dram APs."""
    xt, w1t, w2r, wgt, b1c, b2c, out = (
        io["xt"], io["w1t"], io["w2r"], io["wgt"], io["b1c"], io["b2c"], io["out"],
    )
    with (
        tc.tile_pool(name="main", bufs=1) as pool,
        tc.tile_pool(name="psum", bufs=1, space="PSUM") as psum,
        tc.tile_pool(name="dram", bufs=1, space="DRAM") as dram,
    ):
        # ---- loads: weight chunks first (they gate the collective) ----
        w2_sb = pool.tile([P, E, D], F32)
        HD = D // 2
        for e in range(E):
            for h in range(2):
                nc.sync.dma_start(
                    w2_sb[:, e, h * HD : (h + 1) * HD], w2r[e, :, h * HD : (h + 1) * HD]
                )
        w1_sb = pool.tile([P, E, D], F32)
        DK = D // VK
        for e in range(E):
            for k in range(VK):
                nc.sync.dma_start(
                    w1_sb[:, e, k * DK : (k + 1) * DK], w1t[e, :, k * DK : (k + 1) * DK]
                )
        b1_sb = pool.tile([P, E], F32)
        nc.sync.dma_start(b1_sb[:], b1c)
        b2_sb = pool.tile([1, E * DC], F32)
        nc.sync.dma_start(b2_sb[:], b2c)
        wgt_sb = pool.tile([P, NB * E], F32)
        nc.sync.dma_start(wgt_sb[:], wgt)
        wg3 = wgt_sb.rearrange("p (n e) -> p n e", e=E)

        # ---- phase A: w2sum (chunked reduce), v chunks, payload ----
        w2h = pool.tile([P, 2 * E], F32)
        w2s = pool.tile([P, E], F32)
        for e in range(E):
            for h in range(2):
                nc.vector.reduce_sum(
                    w2h[:, 2 * e + h : 2 * e + h + 1],
                    w2_sb[:, e, h * HD : (h + 1) * HD],
                    axis=AX.X,
                )
            nc.vector.tensor_add(
                w2s[:, e : e + 1], w2h[:, 2 * e : 2 * e + 1], w2h[:, 2 * e + 1 : 2 * e + 2]
            )
        b2s = pool.tile([1, E], F32)
        for e in range(E):
            nc.vector.reduce_sum(
                b2s[0:1, e : e + 1], b2_sb[0:1, e * DC : (e + 1) * DC], axis=AX.X
            )

        pay = pool.tile([1, PAY], F32)
        nc.gpsimd.memset(pay[0:1, 2 * D + 2 : PAY], 0.0)
        b1dot = psum.tile([1, E], F32)
        for e in range(E):
            for k in range(VK):
                vch = psum.tile([1, DK], F32, name="vch", tag="vch", bufs=2)
                nc.tensor.matmul(
                    vch[:],
                    w2s[:, e : e + 1],
                    w1_sb[:, e, k * DK : (k + 1) * DK],
                    start=True,
                    stop=True,
                )
                dst = pay[0:1, e * D + k * DK : e * D + (k + 1) * DK]
                if k % 2 == 0:
                    nc.vector.tensor_copy(dst, vch[:])
                else:
                    nc.scalar.copy(dst, vch[:])
            nc.tensor.matmul(
                b1dot[0:1, e : e + 1],
                w2s[:, e : e + 1],
                b1_sb[:, e : e + 1],
                start=True,
                stop=True,
            )
            nc.vector.tensor_add(
                pay[0:1, 2 * D + e : 2 * D + e + 1],
                b1dot[0:1, e : e + 1],
                b2s[0:1, e : e + 1],
            )

        arin = dram.tile([1, PAY], F32)
        pay_dma = nc.sync.dma_start(arin[:], pay[:])
        if COLLECTIVE == "AG":
            arout = dram.tile([NCORES, PAY], F32, addr_space="Shared")
            nc.gpsimd.collective_compute(
                "AllGather",
                ALU.bypass,
                replica_groups=[list(range(NCORES))],
                ins=[arin[:].opt()],
                outs=[arout[:].opt()],
            )
        else:
            arout = dram.tile([1, PAY], F32, addr_space="Shared")
            nc.gpsimd.collective_compute(
                "AllReduce",
                ALU.add,
                replica_groups=[list(range(NCORES))],
                ins=[arin[:].opt()],
                outs=[arout[:].opt()],
            )

        # ---- x load: FIFO-ordered after the payload store ----
        x_sb = pool.tile([P, NB, TB], F32)
        xv = xt.rearrange("(p n) t -> p n t", p=P)
        x_dmas = []
        for j in range(4):
            x_dmas.append(
                nc.sync.dma_start(x_sb[:, 4 * j : 4 * j + 4, :], xv[:, 4 * j : 4 * j + 4, :])
            )
        for xd in x_dmas:
            bass._add_dep_helper(xd.ins, pay_dma.ins, sync=True, reason="x after payload")
        if F32R_S:
            # rounded copy for the f32r s-stream, made in the collective's shadow
            x_r = pool.tile([P, NB, TB], F32R)
            for j in range(4):
                src = x_sb[:, 4 * j : 4 * j + 4, :]
                dst = x_r[:, 4 * j : 4 * j + 4, :]
                if j % 2 == 0:
                    nc.vector.tensor_copy(dst, src)
                else:
                    nc.scalar.copy(dst, src)

        # ---- logits stream (fp32, runs in the collective's shadow) ----
        lg_ps = psum.tile([E, TB], F32)
        for n in range(NB):
            nc.tensor.matmul(
                lg_ps[:], wg3[:, n, :], x_sb[:, n, :], start=(n == 0), stop=(n == NB - 1)
            )
        sbl = pool.tile([E, TB], F32)
        nc.vector.tensor_copy(sbl[:], lg_ps[:])
        ident2 = pool.tile([E, E], F32)
        make_identity(nc, ident2[:])

        gates, masks = [], []
        for g in range(NG):
            tpl = psum.tile([P, E], F32, name=f"tpl_{g}", tag="tp", bufs=2)
            nc.tensor.transpose(tpl[:], sbl[0:E, g * P : (g + 1) * P], ident2[:])
            t2l = pool.tile([P, E], F32, name=f"t2l_{g}")
            nc.vector.tensor_copy(t2l[:], tpl[:])
            negm = pool.tile([P, 1], F32, name=f"negm_{g}")
            nc.vector.reduce_max(negm[:], t2l[:], axis=AX.X, negate=True)
            z = pool.tile([P, E], F32, name=f"z_{g}")
            den = pool.tile([P, 1], F32, name=f"den_{g}")
            nc.scalar.activation(z[:], t2l[:], AF.Exp, bias=negm[:], accum_out=den[:])
            rec = pool.tile([P, 1], F32, name=f"rec_{g}")
            nc.vector.reciprocal(rec[:], den[:])
            zmax = pool.tile([P, 1], F32, name=f"zmax_{g}")
            nc.vector.reduce_max(zmax[:], z[:], axis=AX.X)
            gate = pool.tile([P, 1], F32, name=f"gate_{g}")
            nc.vector.tensor_mul(gate[:], zmax[:], rec[:])
            mask = pool.tile([P, 1], F32, name=f"mask_{g}")
            nc.vector.tensor_tensor(mask[:], t2l[:, 0:1], t2l[:, 1:2], op=ALU.is_ge)
            gates.append(gate)
            masks.append(mask)

        # ---- combine collective output -> vsb [P, E, NB], csum [1, E] ----
        vsb = pool.tile([P, E, NB], F32)
        csum = pool.tile([1, E], F32)
        if COLLECTIVE == "AG":
            vraw = pool.tile([P, E, NCORES, NB], F32)
            for e in range(E):
                nc.sync.dma_start(
                    vraw[:, e, :, :],
                    arout[:, e * D : (e + 1) * D].rearrange("r (p n) -> p r n", p=P),
                )
            nc.vector.reduce_sum(
                vsb[:], vraw.rearrange("p e r n -> p e n r"), axis=AX.X
            )
            carr = pool.tile([1, E, NCORES], F32)
            nc.sync.dma_start(
                carr[0:1, :, :],
                arout[:, 2 * D : 2 * D + E].rearrange("r c -> c r")[None, :, :],
            )
            nc.vector.reduce_sum(csum[0:1, :], carr[0:1, :, :], axis=AX.X)
        else:
            for e in range(E):
                nc.sync.dma_start(
                    vsb[:, e, :],
                    arout[0:1, e * D : (e + 1) * D].rearrange("x (p n) -> p (x n)", p=P),
                )
            nc.sync.dma_start(csum[:], arout[0:1, 2 * D : 2 * D + E])

        # ---- s stream (f32r ok: nothing discrete depends on s) ----
        ones_row = pool.tile([1, TB], F32)
        nc.gpsimd.memset(ones_row[:], 1.0)
        if F32R_S:
            vsb_r = pool.tile([P, E, NB], F32R)
            nc.vector.tensor_copy(vsb_r[:], vsb[:])
        sg_ps = psum.tile([E, TB], F32)
        for n in range(NB):
            if F32R_S:
                lhs, rhs = vsb_r[:, :, n], x_r[:, n, :]
            else:
                lhs, rhs = vsb[:, :, n], x_sb[:, n, :]
            nc.tensor.matmul(sg_ps[:], lhs, rhs, start=(n == 0), stop=False)
        nc.tensor.matmul(sg_ps[:], csum[0:1, :], ones_row[0:1, :], start=False, stop=True)
        sbs = pool.tile([E, TB], F32)
        nc.vector.tensor_copy(sbs[:], sg_ps[:])

        moe_sb = pool.tile([P, NG], F32)
        for g in range(NG):
            tps = psum.tile([P, E], F32, name=f"tps_{g}", tag="tp", bufs=2)
            nc.tensor.transpose(tps[:], sbs[0:E, g * P : (g + 1) * P], ident2[:])
            t2s = pool.tile([P, E], F32, name=f"t2s_{g}")
            nc.vector.tensor_copy(t2s[:], tps[:])
            sdiff = pool.tile([P, 1], F32, name=f"sdiff_{g}")
            nc.vector.tensor_sub(sdiff[:], t2s[:, 0:1], t2s[:, 1:2])
            ssel = pool.tile([P, 1], F32, name=f"ssel_{g}")
            nc.vector.tensor_mul(ssel[:], masks[g][:], sdiff[:])
            nc.vector.tensor_add(ssel[:], ssel[:], t2s[:, 1:2])
            nc.vector.tensor_mul(moe_sb[:, g : g + 1], gates[g][:], ssel[:])

        # ---- row log_softmax over all 512 tokens ----
        row = pool.tile([1, TB], F32)
        for g in range(NG):
            nc.sync.dma_start(row[0:1, g * P : (g + 1) * P], moe_sb[:, g : g + 1])
        negm2 = pool.tile([1, 1], F32)
        nc.vector.reduce_max(negm2[:], row[0:1, :], axis=AX.X, negate=True)
        expv = pool.tile([1, TB], F32)
        ssum = pool.tile([1, 1], F32)
        nc.scalar.activation(expv[:], row[0:1, :], AF.Exp, bias=negm2[:], accum_out=ssum[:])
        logs = pool.tile([1, 1], F32)
        nc.scalar.activation(logs[:], ssum[:], AF.Ln)
        shift = pool.tile([1, 1], F32)
        nc.vector.tensor_sub(shift[:], negm2[:], logs[:])
        res = pool.tile([1, TB], F32)
        nc.vector.tensor_scalar_add(res[:], row[0:1, :], shift[:])
        nc.sync.dma_start(out[:], res[:])


_CACHED = None


def build_program():
    global _CACHED
    if _CACHED is not None:
        return _CACHED
    nc = bacc.Bacc(
        "TRN2",
        target_bir_lowering=False,
        debug=False,
        enable_asserts=False,
        num_devices=NCORES,
    )
    io = {
        "xt": nc.dram_tensor("xt", [D, TB], F32, kind="ExternalInput").ap(),
        "w1t": nc.dram_tensor("w1t", [E, HC, D], F32, kind="ExternalInput").ap(),
        "w2r": nc.dram_tensor("w2r", [E, HC, D], F32, kind="ExternalInput").ap(),
        "wgt": nc.dram_tensor("wgt", [P, NB * E], F32, kind="ExternalInput").ap(),
        "b1c": nc.dram_tensor("b1c", [HC, E], F32, kind="ExternalInput").ap(),
        "b2c": nc.dram_tensor("b2c", [1, E * DC], F32, kind="ExternalInput").ap(),
        "out": nc.dram_tensor("out", [1, TB], F32, kind="ExternalOutput").ap(),
    }
    with tile.TileContext(nc) as tc:
        emit_kernel(nc, tc, io)
    nc.compile()
    _CACHED = nc
    return nc


def shard_inputs(x, Wg, W1, b1, W2, b2):
    """Host-side sharding: per-core input maps (layout prep only, no math)."""
    x = np.asarray(x, np.float32).reshape(B * T, D)
    Wg = np.asarray(Wg, np.float32)
    W1 = np.asarray(W1, np.float32)
    b1 = np.asarray(b1, np.float32)
    W2 = np.asarray(W2, np.float32)
    b2 = np.asarray(b2, np.float32)
    # wgt[p, n*2+e] = Wg[p*16+n, e]  (d = p*16 + n decomposition)
    wgt = np.ascontiguousarray(Wg.reshape(P, NB * E))
    in_maps = []
    for c in range(NCORES):
        row = c % B
        hs, he = c * HC, (c + 1) * HC
        in_maps.append(
            {
                "xt": np.ascontiguousarray(x[row * TB : (row + 1) * TB, :].T),
                "w1t": np.ascontiguousarray(W1[:, :, hs:he].transpose(0, 2, 1)),
                "w2r": np.ascontiguousarray(W2[:, hs:he, :]),
                "wgt": wgt,
                "b1c": np.ascontiguousarray(b1[:, hs:he].T),
                "b2c": np.ascontiguousarray(
                    b2[:, c * DC : (c + 1) * DC].reshape(1, E * DC)
                ),
            }
        )
    return in_maps


def run(in_maps, **kwargs):
    nc = build_program()
    return bass_utils.run_bass_kernel_spmd(
        nc, in_maps, core_ids=list(range(NCORES)), **kwargs
    )


def kernel(x, Wg, W1, b1, W2, b2):
    res = run(shard_inputs(x, Wg, W1, b1, W2, b2))
    return np.concatenate([res.results[b]["out"] for b in range(B)], axis=0)
